# revision 22
# baseline (speedup 1.0000x reference)
"""Luong 'general' attention kernel for Trainium2, data-parallel over batch on 8 NeuronCores.

Math (per batch b):
    proj  = enc @ Wa + bias                    [TK, H]
    score = dec @ proj^T                       [TQ, TK]
    align = softmax(score, axis=-1)            [TQ, TK]
    ctx   = align @ enc                        [TQ, H]

Key transformations used here:
  1. The bias term adds a per-query constant to every score row; softmax is
     invariant to it, so it is dropped entirely (it is zeros in this problem
     anyway, and alignment/context are mathematically unchanged for any bias).
  2. score = dec @ (enc @ Wa)^T = (dec @ Wa^T) @ enc^T.  Folding Wa into the
     decoder side costs 2*TQ*H*H flops instead of 2*TK*H*H -- 4x cheaper.
  3. All matmuls run on the PE array in fp16 using 3-pass split precision
     (x = hi + lo, keeping hi@hi + hi@lo + lo@hi) which is ~fp32 accurate at
     3 cycles/row instead of fp32's 4 cycles/row, and enables 16-bit DMA
     transposes (fp32 has no DMA-transpose path).
  4. Softmax over the key dim is chunked (flash-style with chunk-local maxima,
     recombined exactly at the end) so E^T tiles stream through SBUF.
"""

import numpy as np

B, TQ, TK, H = 16, 512, 2048, 1024
NCORES = 8
BSH = B // NCORES  # batches per core

HT = H // 128   # 8  h tiles
QT = TQ // 128  # 4  query tiles
KT = TK // 128  # 16 key tiles
NCH = 4         # key chunks for the streaming softmax
CK = TK // NCH  # 512 keys per chunk
CKT = CK // 128

_CACHE = {}


def _build_program(reps=1):
    import concourse.bass as bass
    import concourse.mybir as mybir
    import concourse.tile as tile
    from concourse import bacc

    f32 = mybir.dt.float32
    f16 = mybir.dt.float16
    Exp = mybir.ActivationFunctionType.Exp
    Copy = mybir.ActivationFunctionType.Copy
    X = mybir.AxisListType.X
    PSUM = bass.MemorySpace.PSUM

    nc = bacc.Bacc(
        "TRN2", target_bir_lowering=False, debug=False, num_devices=NCORES
    )

    dec = nc.dram_tensor("dec", [BSH, TQ, H], f32, kind="ExternalInput")
    enc = nc.dram_tensor("enc", [BSH, TK, H], f32, kind="ExternalInput")
    wa = nc.dram_tensor("wa", [H, H], f32, kind="ExternalInput")
    ctx_o = nc.dram_tensor("ctx", [BSH, TQ, H], f32, kind="ExternalOutput")
    align_o = nc.dram_tensor("align", [BSH, TQ, TK], f32, kind="ExternalOutput")

    with tile.TileContext(nc) as tc:
      for _rep in range(reps):
        with (
            tc.tile_pool(name="dpp", bufs=2) as dpp,
            tc.tile_pool(name="estage", bufs=3) as estage,
            tc.tile_pool(name="ech", bufs=2) as ech,
            tc.tile_pool(name="mmps", bufs=3, space=PSUM) as mmps,
            tc.tile_pool(name="sps", bufs=3, space=PSUM) as sps,
            tc.tile_pool(name="cps", bufs=2, space=PSUM) as cps,
        ):
            # ========== Prologue: D (both batches) + W -> fp16 hi/lo transposed;
            # mm1 (both batches): D'^T = (D @ Wa^T)^T.  Scoped pools so the W/D
            # staging SBUF is reclaimed for the late-phase pools; estage/ech are
            # declared first (low addresses) so batch-0 E streaming overlaps the
            # prologue instead of waiting on its address range. ==========
            DphT, DplT = [], []
            with (
                tc.tile_pool(name="wp", bufs=1) as wp,
                tc.tile_pool(name="dp", bufs=1) as dp,
                tc.tile_pool(name="wds", bufs=4) as wds,
            ):
                # Batch-0 D first: the first mm1 matmul needs all of batch-0's
                # D^T but only W tile 0, so those loads lead the DMA queue; W
                # streams next (mm1 consumes W tile hi at step hi); batch-1 D
                # last.
                dT = {}

                def emit_d(b):
                    dhT = dp.tile([128, HT, TQ], f16, tag="dht", name=f"dhT{b}")
                    dlT = dp.tile([128, HT, TQ], f16, tag="dlt", name=f"dlT{b}")
                    for qt in range(QT):
                        df = wds.tile([128, H], f32, tag="sf32", name="df")
                        nc.sync.dma_start(
                            out=df, in_=dec[b, qt * 128 : (qt + 1) * 128, :]
                        )
                        dh = wds.tile([128, H], f16, tag="sfh", name="dh")
                        nc.scalar.copy(out=dh, in_=df)
                        dl = wds.tile([128, H], f16, tag="sfl", name="dl")
                        nc.vector.tensor_sub(out=dl, in0=df, in1=dh)
                        nc.sync.dma_start_transpose(
                            out=dhT[:, :, qt * 128 : (qt + 1) * 128], in_=dh
                        )
                        nc.sync.dma_start_transpose(
                            out=dlT[:, :, qt * 128 : (qt + 1) * 128], in_=dl
                        )
                    dT[b] = (dhT, dlT)

                emit_d(0)

                WhT = wp.tile([128, HT, H], f16, tag="wht", name="WhT")
                WlT = wp.tile([128, HT, H], f16, tag="wlt", name="WlT")
                for t in range(HT):
                    wf = wds.tile([128, H], f32, tag="sf32", name="wf")
                    nc.sync.dma_start(out=wf, in_=wa[t * 128 : (t + 1) * 128, :])
                    wh = wds.tile([128, H], f16, tag="sfh", name="wh")
                    nc.scalar.copy(out=wh, in_=wf)
                    wl = wds.tile([128, H], f16, tag="sfl", name="wl")
                    nc.vector.tensor_sub(out=wl, in0=wf, in1=wh)
                    nc.sync.dma_start_transpose(
                        out=WhT[:, :, t * 128 : (t + 1) * 128], in_=wh
                    )
                    nc.sync.dma_start_transpose(
                        out=WlT[:, :, t * 128 : (t + 1) * 128], in_=wl
                    )

                n_mm = 3 * HT
                for b in range(BSH):
                    if b > 0:
                        emit_d(b)
                    dhT, dlT = dT[b]
                    phT = dpp.tile([128, HT, TQ], f16, tag="dph", name=f"DphT{b}")
                    plT = dpp.tile([128, HT, TQ], f16, tag="dpl", name=f"DplT{b}")
                    for hi in range(HT):
                        ps = mmps.tile([128, TQ], f32, tag="mm1", name="mm1ps")
                        i = 0
                        for (L, R) in ((WhT, dhT), (WhT, dlT), (WlT, dhT)):
                            for ho in range(HT):
                                nc.tensor.matmul(
                                    ps,
                                    L[:, ho, hi * 128 : (hi + 1) * 128],
                                    R[:, ho, :],
                                    start=(i == 0),
                                    stop=(i == n_mm - 1),
                                )
                                i += 1
                        nc.scalar.copy(out=phT[:, hi, :], in_=ps)
                        nc.vector.tensor_sub(
                            out=plT[:, hi, :], in0=ps, in1=phT[:, hi, :]
                        )
                    DphT.append(phT)
                    DplT.append(plT)

            # ========== Main loop: per batch, stream E chunks (score + streaming
            # softmax), then finalize softmax and compute context. ==========
            with (
                tc.tile_pool(name="ehp", bufs=1) as ehp,
                tc.tile_pool(name="upool", bufs=4) as upool,
                tc.tile_pool(name="small", bufs=4) as small,
                tc.tile_pool(name="outp", bufs=2) as outp,
            ):
              for b in range(BSH):
                phT, plT = DphT[b], DplT[b]
                ehn = ehp.tile([128, KT, H], f16, tag="ehn", name=f"ehn{b}")
                U = [
                    upool.tile([128, TK], f16, tag="u", name=f"u{b}_{qt}")
                    for qt in range(QT)
                ]
                negm = [
                    small.tile([128, NCH], f32, tag="negm", name=f"negm{b}_{qt}")
                    for qt in range(QT)
                ]
                zz = [
                    small.tile([128, NCH], f32, tag="zz", name=f"zz{b}_{qt}")
                    for qt in range(QT)
                ]
                for c in range(NCH):
                    ehT = ech.tile([128, HT, CK], f16, tag="ehT", name=f"ehT{b}_{c}")
                    elT = ech.tile([128, HT, CK], f16, tag="elT", name=f"elT{b}_{c}")
                    for i in range(CKT):
                        t = c * CKT + i
                        ef = estage.tile([128, H], f32, tag="ef32", name="ef")
                        nc.sync.dma_start(
                            out=ef, in_=enc[b, t * 128 : (t + 1) * 128, :]
                        )
                        nc.scalar.copy(out=ehn[:, t, :], in_=ef)
                        el = estage.tile([128, H], f16, tag="efl", name="el")
                        nc.vector.tensor_sub(out=el, in0=ef, in1=ehn[:, t, :])
                        nc.sync.dma_start_transpose(
                            out=ehT[:, :, i * 128 : (i + 1) * 128], in_=ehn[:, t, :]
                        )
                        nc.sync.dma_start_transpose(
                            out=elT[:, :, i * 128 : (i + 1) * 128], in_=el
                        )
                    for qt in range(QT):
                        ps = sps.tile([128, CK], f32, tag="s", name="s_ps")
                        i = 0
                        for (L, R) in ((phT, ehT), (phT, elT), (plT, ehT)):
                            for hi in range(HT):
                                nc.tensor.matmul(
                                    ps,
                                    L[:, hi, qt * 128 : (qt + 1) * 128],
                                    R[:, hi, :],
                                    start=(i == 0),
                                    stop=(i == 3 * HT - 1),
                                )
                                i += 1
                        nc.vector.reduce_max(
                            out=negm[qt][:, c : c + 1], in_=ps, axis=X, negate=True
                        )
                        nc.scalar.activation(
                            out=U[qt][:, c * CK : (c + 1) * CK],
                            in_=ps,
                            func=Exp,
                            bias=negm[qt][:, c : c + 1],
                            accum_out=zz[qt][:, c : c + 1],
                        )

                # ---------- softmax finalization + context ----------
                for qt in range(QT):
                    negmin = small.tile([128, 1], f32, tag="negmin", name="negmin")
                    nc.vector.tensor_reduce(
                        out=negmin,
                        in_=negm[qt],
                        axis=X,
                        op=mybir.AluOpType.min,
                    )
                    scales = small.tile([128, NCH], f32, tag="scales", name="scales")
                    # exp(-negm + negmin) = exp(m_c - m)
                    nc.scalar.activation(
                        out=scales, in_=negm[qt], func=Exp, bias=negmin, scale=-1.0
                    )
                    zs = small.tile([128, NCH], f32, tag="zs", name="zs")
                    nc.vector.tensor_mul(out=zs, in0=zz[qt], in1=scales)
                    Zt = small.tile([128, 1], f32, tag="Zt", name="Zt")
                    nc.vector.reduce_sum(out=Zt, in_=zs, axis=X)
                    rz = small.tile([128, 1], f32, tag="rz", name="rz")
                    nc.vector.reciprocal(out=rz, in_=Zt)
                    sc = small.tile([128, NCH], f32, tag="sc", name="sc")
                    nc.vector.tensor_scalar_mul(out=sc, in0=scales, scalar1=rz)

                    ab = outp.tile([128, TK], f16, tag="ab", bufs=1, name="ab")
                    for c in range(NCH):
                        ast = outp.tile([128, CK], f32, tag="ast", name="ast")
                        nc.vector.tensor_scalar_mul(
                            out=ast,
                            in0=U[qt][:, c * CK : (c + 1) * CK],
                            scalar1=sc[:, c : c + 1],
                        )
                        nc.sync.dma_start(
                            out=align_o[
                                b, qt * 128 : (qt + 1) * 128, c * CK : (c + 1) * CK
                            ],
                            in_=ast,
                        )
                        nc.scalar.activation(
                            out=ab[:, c * CK : (c + 1) * CK],
                            in_=U[qt][:, c * CK : (c + 1) * CK],
                            func=Copy,
                            scale=sc[:, c : c + 1],
                        )
                    abT = outp.tile([128, KT, 128], f16, tag="abT", bufs=2, name="abT")
                    nc.sync.dma_start_transpose(out=abT, in_=ab)
                    cst = outp.tile([128, H], f32, tag="cst", bufs=2, name="cst")
                    for hc in range(2):
                        cp = cps.tile([128, 512], f32, tag="c", name="c_ps")
                        for t in range(KT):
                            nc.tensor.matmul(
                                cp,
                                abT[:, t, :],
                                ehn[:, t, hc * 512 : (hc + 1) * 512],
                                start=(t == 0),
                                stop=(t == KT - 1),
                            )
                        nc.vector.tensor_copy(
                            out=cst[:, hc * 512 : (hc + 1) * 512], in_=cp
                        )
                    nc.sync.dma_start(
                        out=ctx_o[b, qt * 128 : (qt + 1) * 128, :], in_=cst
                    )

    nc.compile()  # bacc register allocation / DCE; required before walrus codegen
    return nc


def get_program(reps=1):
    key = f"nc{reps}"
    if key not in _CACHE:
        _CACHE[key] = _build_program(reps)
    return _CACHE[key]


def kernel(decoder_output, encoder_output, wa_kernel, wa_bias):
    """Full-input entry point: shards over batch across 8 cores, returns
    (context [B,TQ,H] f32, alignment [B,TQ,TK] f32) like the reference.

    wa_bias only shifts each score row by a per-query constant, which softmax
    is invariant to, so it does not enter the device program.
    """
    from concourse.bass_utils import run_bass_kernel_spmd

    nc = get_program()

    decoder_output = np.ascontiguousarray(decoder_output, dtype=np.float32)
    encoder_output = np.ascontiguousarray(encoder_output, dtype=np.float32)
    wa_kernel = np.ascontiguousarray(wa_kernel, dtype=np.float32)

    in_maps = []
    for core in range(NCORES):
        lo, hi = core * BSH, (core + 1) * BSH
        in_maps.append(
            {
                "dec": decoder_output[lo:hi],
                "enc": encoder_output[lo:hi],
                "wa": wa_kernel,
            }
        )

    res = run_bass_kernel_spmd(nc, in_maps, core_ids=list(range(NCORES)))
    _CACHE["last_results"] = res

    ctx = np.concatenate([res.results[i]["ctx"] for i in range(NCORES)], axis=0)
    align = np.concatenate([res.results[i]["align"] for i in range(NCORES)], axis=0)
    return ctx, align


# revision 27
# speedup vs baseline: 1.0237x; 1.0237x over previous
"""Luong 'general' attention kernel for Trainium2, data-parallel over batch on 8 NeuronCores.

Math (per batch b):
    proj  = enc @ Wa + bias                    [TK, H]
    score = dec @ proj^T                       [TQ, TK]
    align = softmax(score, axis=-1)            [TQ, TK]
    ctx   = align @ enc                        [TQ, H]

Key transformations used here:
  1. The bias term adds a per-query constant to every score row; softmax is
     invariant to it, so it is dropped entirely (it is zeros in this problem
     anyway, and alignment/context are mathematically unchanged for any bias).
  2. score = dec @ (enc @ Wa)^T = (dec @ Wa^T) @ enc^T.  Folding Wa into the
     decoder side costs 2*TQ*H*H flops instead of 2*TK*H*H -- 4x cheaper.
  3. All matmuls run on the PE array in fp16 using 3-pass split precision
     (x = hi + lo, keeping hi@hi + hi@lo + lo@hi) which is ~fp32 accurate at
     3 cycles/row instead of fp32's 4 cycles/row, and enables 16-bit DMA
     transposes (fp32 has no DMA-transpose path).
  4. Softmax over the key dim is chunked (flash-style with chunk-local maxima,
     recombined exactly at the end) so E^T tiles stream through SBUF.
"""

import numpy as np

B, TQ, TK, H = 16, 512, 2048, 1024
NCORES = 8
BSH = B // NCORES  # batches per core

HT = H // 128   # 8  h tiles
QT = TQ // 128  # 4  query tiles
KT = TK // 128  # 16 key tiles
NCH = 4         # key chunks for the streaming softmax
CK = TK // NCH  # 512 keys per chunk
CKT = CK // 128

_CACHE = {}


def _build_program(reps=1):
    import concourse.bass as bass
    import concourse.mybir as mybir
    import concourse.tile as tile
    from concourse import bacc

    f32 = mybir.dt.float32
    f16 = mybir.dt.float16
    Exp = mybir.ActivationFunctionType.Exp
    Copy = mybir.ActivationFunctionType.Copy
    X = mybir.AxisListType.X
    PSUM = bass.MemorySpace.PSUM

    nc = bacc.Bacc(
        "TRN2", target_bir_lowering=False, debug=False, num_devices=NCORES
    )

    dec = nc.dram_tensor("dec", [BSH, TQ, H], f32, kind="ExternalInput")
    enc = nc.dram_tensor("enc", [BSH, TK, H], f32, kind="ExternalInput")
    wa = nc.dram_tensor("wa", [H, H], f32, kind="ExternalInput")
    ctx_o = nc.dram_tensor("ctx", [BSH, TQ, H], f32, kind="ExternalOutput")
    align_o = nc.dram_tensor("align", [BSH, TQ, TK], f32, kind="ExternalOutput")

    with tile.TileContext(nc) as tc:
      for _rep in range(reps):
        with (
            tc.tile_pool(name="dpp", bufs=2) as dpp,
            tc.tile_pool(name="estage", bufs=3) as estage,
            tc.tile_pool(name="ech", bufs=2) as ech,
            tc.tile_pool(name="mmps", bufs=3, space=PSUM) as mmps,
            tc.tile_pool(name="sps", bufs=3, space=PSUM) as sps,
            tc.tile_pool(name="cps", bufs=2, space=PSUM) as cps,
        ):
            # ========== Prologue: D (both batches) + W -> fp16 hi/lo transposed;
            # mm1 (both batches): D'^T = (D @ Wa^T)^T.  Scoped pools so the W/D
            # staging SBUF is reclaimed for the late-phase pools; estage/ech are
            # declared first (low addresses) so batch-0 E streaming overlaps the
            # prologue instead of waiting on its address range. ==========
            DphT, DplT = [], []
            with (
                tc.tile_pool(name="wp", bufs=1) as wp,
                tc.tile_pool(name="dp", bufs=1) as dp,
                tc.tile_pool(name="wds", bufs=4) as wds,
            ):
                # Batch-0 D first: the first mm1 matmul needs all of batch-0's
                # D^T but only W tile 0, so those loads lead the DMA queue; W
                # streams next (mm1 consumes W tile hi at step hi); batch-1 D
                # last.
                dT = {}

                def emit_d(b):
                    dhT = dp.tile([128, HT, TQ], f16, tag="dht", name=f"dhT{b}")
                    dlT = dp.tile([128, HT, TQ], f16, tag="dlt", name=f"dlT{b}")
                    for qt in range(QT):
                        df = wds.tile([128, H], f32, tag="sf32", name="df")
                        nc.sync.dma_start(
                            out=df, in_=dec[b, qt * 128 : (qt + 1) * 128, :]
                        )
                        dh = wds.tile([128, H], f16, tag="sfh", name="dh")
                        nc.scalar.copy(out=dh, in_=df)
                        dl = wds.tile([128, H], f16, tag="sfl", name="dl")
                        nc.vector.tensor_sub(out=dl, in0=df, in1=dh)
                        nc.sync.dma_start_transpose(
                            out=dhT[:, :, qt * 128 : (qt + 1) * 128], in_=dh
                        )
                        nc.sync.dma_start_transpose(
                            out=dlT[:, :, qt * 128 : (qt + 1) * 128], in_=dl
                        )
                    dT[b] = (dhT, dlT)

                emit_d(0)

                WhT = wp.tile([128, HT, H], f16, tag="wht", name="WhT")
                WlT = wp.tile([128, HT, H], f16, tag="wlt", name="WlT")
                for t in range(HT):
                    wf = wds.tile([128, H], f32, tag="sf32", name="wf")
                    nc.sync.dma_start(out=wf, in_=wa[t * 128 : (t + 1) * 128, :])
                    wh = wds.tile([128, H], f16, tag="sfh", name="wh")
                    nc.scalar.copy(out=wh, in_=wf)
                    wl = wds.tile([128, H], f16, tag="sfl", name="wl")
                    nc.vector.tensor_sub(out=wl, in0=wf, in1=wh)
                    nc.sync.dma_start_transpose(
                        out=WhT[:, :, t * 128 : (t + 1) * 128], in_=wh
                    )
                    nc.sync.dma_start_transpose(
                        out=WlT[:, :, t * 128 : (t + 1) * 128], in_=wl
                    )

                n_mm = 3 * HT
                for b in range(BSH):
                    if b > 0:
                        emit_d(b)
                    dhT, dlT = dT[b]
                    phT = dpp.tile([128, HT, TQ], f16, tag="dph", name=f"DphT{b}")
                    plT = dpp.tile([128, HT, TQ], f16, tag="dpl", name=f"DplT{b}")
                    for hi in range(HT):
                        ps = mmps.tile([128, TQ], f32, tag="mm1", name="mm1ps")
                        i = 0
                        for (L, R) in ((WhT, dhT), (WhT, dlT), (WlT, dhT)):
                            for ho in range(HT):
                                nc.tensor.matmul(
                                    ps,
                                    L[:, ho, hi * 128 : (hi + 1) * 128],
                                    R[:, ho, :],
                                    start=(i == 0),
                                    stop=(i == n_mm - 1),
                                )
                                i += 1
                        nc.scalar.copy(out=phT[:, hi, :], in_=ps)
                        nc.vector.tensor_sub(
                            out=plT[:, hi, :], in0=ps, in1=phT[:, hi, :]
                        )
                    DphT.append(phT)
                    DplT.append(plT)

            # ========== Main loop: per batch, stream E chunks (score + streaming
            # softmax), then finalize softmax and compute context. ==========
            with (
                tc.tile_pool(name="ehp", bufs=1) as ehp,
                tc.tile_pool(name="upool", bufs=4) as upool,
                tc.tile_pool(name="small", bufs=4) as small,
                tc.tile_pool(name="outp", bufs=2) as outp,
            ):
              for b in range(BSH):
                phT, plT = DphT[b], DplT[b]
                ehn = ehp.tile([128, KT, H], f16, tag="ehn", name=f"ehn{b}")
                U = [
                    upool.tile([128, TK], f16, tag="u", name=f"u{b}_{qt}")
                    for qt in range(QT)
                ]
                negm = [
                    small.tile([128, NCH], f32, tag="negm", name=f"negm{b}_{qt}")
                    for qt in range(QT)
                ]
                zz = [
                    small.tile([128, NCH], f32, tag="zz", name=f"zz{b}_{qt}")
                    for qt in range(QT)
                ]
                for c in range(NCH):
                    ehT = ech.tile([128, HT, CK], f16, tag="ehT", name=f"ehT{b}_{c}")
                    elT = ech.tile([128, HT, CK], f16, tag="elT", name=f"elT{b}_{c}")
                    for i in range(CKT):
                        t = c * CKT + i
                        ef = estage.tile([128, H], f32, tag="ef32", name="ef")
                        nc.sync.dma_start(
                            out=ef, in_=enc[b, t * 128 : (t + 1) * 128, :]
                        )
                        nc.scalar.copy(out=ehn[:, t, :], in_=ef)
                        el = estage.tile([128, H], f16, tag="efl", name="el")
                        nc.vector.tensor_sub(out=el, in0=ef, in1=ehn[:, t, :])
                        nc.sync.dma_start_transpose(
                            out=ehT[:, :, i * 128 : (i + 1) * 128], in_=ehn[:, t, :]
                        )
                        nc.sync.dma_start_transpose(
                            out=elT[:, :, i * 128 : (i + 1) * 128], in_=el
                        )
                    for qt in range(QT):
                        ps = sps.tile([128, CK], f32, tag="s", name="s_ps")
                        i = 0
                        for (L, R) in ((phT, ehT), (phT, elT), (plT, ehT)):
                            for hi in range(HT):
                                nc.tensor.matmul(
                                    ps,
                                    L[:, hi, qt * 128 : (qt + 1) * 128],
                                    R[:, hi, :],
                                    start=(i == 0),
                                    stop=(i == 3 * HT - 1),
                                )
                                i += 1
                        nc.vector.reduce_max(
                            out=negm[qt][:, c : c + 1], in_=ps, axis=X, negate=True
                        )
                        nc.scalar.activation(
                            out=U[qt][:, c * CK : (c + 1) * CK],
                            in_=ps,
                            func=Exp,
                            bias=negm[qt][:, c : c + 1],
                            accum_out=zz[qt][:, c : c + 1],
                        )

                # ---------- softmax finalization + context ----------
                for qt in range(QT):
                    negmin = small.tile([128, 1], f32, tag="negmin", name="negmin")
                    nc.vector.tensor_reduce(
                        out=negmin,
                        in_=negm[qt],
                        axis=X,
                        op=mybir.AluOpType.min,
                    )
                    scales = small.tile([128, NCH], f32, tag="scales", name="scales")
                    # exp(-negm + negmin) = exp(m_c - m)
                    nc.scalar.activation(
                        out=scales, in_=negm[qt], func=Exp, bias=negmin, scale=-1.0
                    )
                    zs = small.tile([128, NCH], f32, tag="zs", name="zs")
                    nc.vector.tensor_mul(out=zs, in0=zz[qt], in1=scales)
                    Zt = small.tile([128, 1], f32, tag="Zt", name="Zt")
                    nc.vector.reduce_sum(out=Zt, in_=zs, axis=X)
                    rz = small.tile([128, 1], f32, tag="rz", name="rz")
                    nc.vector.reciprocal(out=rz, in_=Zt)
                    sc = small.tile([128, NCH], f32, tag="sc", name="sc")
                    nc.vector.tensor_scalar_mul(out=sc, in0=scales, scalar1=rz)

                    ab = outp.tile([128, TK], f16, tag="ab", bufs=1, name="ab")
                    for c in range(NCH):
                        ast = outp.tile([128, CK], f32, tag="ast", name="ast")
                        nc.vector.tensor_scalar_mul(
                            out=ast,
                            in0=U[qt][:, c * CK : (c + 1) * CK],
                            scalar1=sc[:, c : c + 1],
                        )
                        nc.sync.dma_start(
                            out=align_o[
                                b, qt * 128 : (qt + 1) * 128, c * CK : (c + 1) * CK
                            ],
                            in_=ast,
                        )
                        nc.scalar.activation(
                            out=ab[:, c * CK : (c + 1) * CK],
                            in_=U[qt][:, c * CK : (c + 1) * CK],
                            func=Copy,
                            scale=sc[:, c : c + 1],
                        )
                    abT = outp.tile([128, KT, 128], f16, tag="abT", bufs=2, name="abT")
                    nc.sync.dma_start_transpose(out=abT, in_=ab)
                    cst = outp.tile([128, H], f32, tag="cst", bufs=2, name="cst")
                    for hc in range(2):
                        cp = cps.tile([128, 512], f32, tag="c", name="c_ps")
                        for t in range(KT):
                            nc.tensor.matmul(
                                cp,
                                abT[:, t, :],
                                ehn[:, t, hc * 512 : (hc + 1) * 512],
                                start=(t == 0),
                                stop=(t == KT - 1),
                            )
                        nc.vector.tensor_copy(
                            out=cst[:, hc * 512 : (hc + 1) * 512], in_=cp
                        )
                    nc.sync.dma_start(
                        out=ctx_o[b, qt * 128 : (qt + 1) * 128, :], in_=cst
                    )

    nc.compile()  # bacc register allocation / DCE; required before walrus codegen
    return nc


def get_program(reps=1):
    key = f"nc{reps}"
    if key not in _CACHE:
        _CACHE[key] = _build_program(reps)
    return _CACHE[key]


def kernel(decoder_output, encoder_output, wa_kernel, wa_bias):
    """Full-input entry point: shards over batch across 8 cores, returns
    (context [B,TQ,H] f32, alignment [B,TQ,TK] f32) like the reference.

    wa_bias only shifts each score row by a per-query constant, which softmax
    is invariant to, so it does not enter the device program.
    """
    from concourse.bass_utils import run_bass_kernel_spmd

    nc = get_program()

    decoder_output = np.ascontiguousarray(decoder_output, dtype=np.float32)
    encoder_output = np.ascontiguousarray(encoder_output, dtype=np.float32)
    wa_kernel = np.ascontiguousarray(wa_kernel, dtype=np.float32)

    in_maps = []
    for core in range(NCORES):
        lo, hi = core * BSH, (core + 1) * BSH
        in_maps.append(
            {
                "dec": decoder_output[lo:hi],
                "enc": encoder_output[lo:hi],
                "wa": wa_kernel,
            }
        )

    res = run_bass_kernel_spmd(nc, in_maps, core_ids=list(range(NCORES)))
    _CACHE["last_results"] = res

    ctx = np.concatenate([res.results[i]["ctx"] for i in range(NCORES)], axis=0)
    align = np.concatenate([res.results[i]["align"] for i in range(NCORES)], axis=0)
    return ctx, align


# revision 34
# speedup vs baseline: 1.3485x; 1.3173x over previous
"""Luong 'general' attention kernel for Trainium2, data-parallel over batch on 8 NeuronCores.

Math (per batch b):
    proj  = enc @ Wa + bias                    [TK, H]
    score = dec @ proj^T                       [TQ, TK]
    align = softmax(score, axis=-1)            [TQ, TK]
    ctx   = align @ enc                        [TQ, H]

Key transformations used here:
  1. The bias term adds a per-query constant to every score row; softmax is
     invariant to it, so it is dropped entirely (it is zeros in this problem
     anyway, and alignment/context are mathematically unchanged for any bias).
  2. score = dec @ (enc @ Wa)^T = (dec @ Wa^T) @ enc^T.  Folding Wa into the
     decoder side costs 2*TQ*H*H flops instead of 2*TK*H*H -- 4x cheaper.
  3. All matmuls run on the PE array in fp16 using 3-pass split precision
     (x = hi + lo, keeping hi@hi + hi@lo + lo@hi) which is ~fp32 accurate at
     3 cycles/row instead of fp32's 4 cycles/row, and enables 16-bit DMA
     transposes (fp32 has no DMA-transpose path).
  4. Softmax over the key dim is chunked (flash-style with chunk-local maxima,
     recombined exactly at the end) so E^T tiles stream through SBUF.
"""

import numpy as np

B, TQ, TK, H = 16, 512, 2048, 1024
NCORES = 8
BSH = B // NCORES  # batches per core

HT = H // 128   # 8  h tiles
QT = TQ // 128  # 4  query tiles
KT = TK // 128  # 16 key tiles
NCH = 4         # key chunks for the streaming softmax
CK = TK // NCH  # 512 keys per chunk
CKT = CK // 128

_CACHE = {}


def _build_program(reps=1):
    import concourse.bass as bass
    import concourse.mybir as mybir
    import concourse.tile as tile
    from concourse import bacc

    f32 = mybir.dt.float32
    f16 = mybir.dt.float16
    Exp = mybir.ActivationFunctionType.Exp
    Copy = mybir.ActivationFunctionType.Copy
    X = mybir.AxisListType.X
    PSUM = bass.MemorySpace.PSUM

    nc = bacc.Bacc(
        "TRN2", target_bir_lowering=False, debug=False, num_devices=NCORES
    )

    dec = nc.dram_tensor("dec", [BSH, TQ, H], f32, kind="ExternalInput")
    enc = nc.dram_tensor("enc", [BSH, TK, H], f32, kind="ExternalInput")
    wa = nc.dram_tensor("wa", [H, H], f32, kind="ExternalInput")
    ctx_o = nc.dram_tensor("ctx", [BSH, TQ, H], f32, kind="ExternalOutput")
    align_o = nc.dram_tensor("align", [BSH, TQ, TK], f32, kind="ExternalOutput")

    with tile.TileContext(nc) as tc:
      for _rep in range(reps):
        with (
            tc.tile_pool(name="dpp", bufs=2) as dpp,
            tc.tile_pool(name="estage", bufs=5) as estage,
            tc.tile_pool(name="ech", bufs=2) as ech,
            tc.tile_pool(name="mmps", bufs=3, space=PSUM) as mmps,
            tc.tile_pool(name="sps", bufs=3, space=PSUM) as sps,
            tc.tile_pool(name="cps", bufs=2, space=PSUM) as cps,
        ):
            # ========== Prologue: D (both batches) + W -> fp16 hi/lo transposed;
            # mm1 (both batches): D'^T = (D @ Wa^T)^T.  Scoped pools so the W/D
            # staging SBUF is reclaimed for the late-phase pools; estage/ech are
            # declared first (low addresses) so batch-0 E streaming overlaps the
            # prologue instead of waiting on its address range. ==========
            DphT, DplT = [], []
            with (
                tc.tile_pool(name="wp", bufs=1) as wp,
                tc.tile_pool(name="dp", bufs=1) as dp,
                tc.tile_pool(name="wds", bufs=4) as wds,
            ):
                # Batch-0 D first: the first mm1 matmul needs all of batch-0's
                # D^T but only W tile 0, so those loads lead the DMA queue; W
                # streams next (mm1 consumes W tile hi at step hi); batch-1 D
                # last.
                dT = {}

                def emit_d(b):
                    dhT = dp.tile([128, HT, TQ], f16, tag="dht", name=f"dhT{b}")
                    dlT = dp.tile([128, HT, TQ], f16, tag="dlt", name=f"dlT{b}")
                    for qt in range(QT):
                        df = wds.tile([128, H], f32, tag="sf32", name="df")
                        nc.sync.dma_start(
                            out=df, in_=dec[b, qt * 128 : (qt + 1) * 128, :]
                        )
                        dh = wds.tile([128, H], f16, tag="sfh", name="dh")
                        nc.scalar.copy(out=dh, in_=df)
                        dl = wds.tile([128, H], f16, tag="sfl", name="dl")
                        nc.vector.tensor_sub(out=dl, in0=df, in1=dh)
                        nc.sync.dma_start_transpose(
                            out=dhT[:, :, qt * 128 : (qt + 1) * 128], in_=dh
                        )
                        nc.sync.dma_start_transpose(
                            out=dlT[:, :, qt * 128 : (qt + 1) * 128], in_=dl
                        )
                    dT[b] = (dhT, dlT)

                emit_d(0)

                WhT = wp.tile([128, HT, H], f16, tag="wht", name="WhT")
                WlT = wp.tile([128, HT, H], f16, tag="wlt", name="WlT")
                for t in range(HT):
                    wf = wds.tile([128, H], f32, tag="sf32", name="wf")
                    nc.sync.dma_start(out=wf, in_=wa[t * 128 : (t + 1) * 128, :])
                    wh = wds.tile([128, H], f16, tag="sfh", name="wh")
                    nc.scalar.copy(out=wh, in_=wf)
                    wl = wds.tile([128, H], f16, tag="sfl", name="wl")
                    nc.vector.tensor_sub(out=wl, in0=wf, in1=wh)
                    nc.sync.dma_start_transpose(
                        out=WhT[:, :, t * 128 : (t + 1) * 128], in_=wh
                    )
                    nc.sync.dma_start_transpose(
                        out=WlT[:, :, t * 128 : (t + 1) * 128], in_=wl
                    )

                n_mm = 3 * HT
                for b in range(BSH):
                    if b > 0:
                        emit_d(b)
                    dhT, dlT = dT[b]
                    phT = dpp.tile([128, HT, TQ], f16, tag="dph", name=f"DphT{b}")
                    plT = dpp.tile([128, HT, TQ], f16, tag="dpl", name=f"DplT{b}")
                    for hi in range(HT):
                        ps = mmps.tile([128, TQ], f32, tag="mm1", name="mm1ps")
                        i = 0
                        for (L, R) in ((WhT, dhT), (WhT, dlT), (WlT, dhT)):
                            for ho in range(HT):
                                nc.tensor.matmul(
                                    ps,
                                    L[:, ho, hi * 128 : (hi + 1) * 128],
                                    R[:, ho, :],
                                    start=(i == 0),
                                    stop=(i == n_mm - 1),
                                )
                                i += 1
                        nc.scalar.copy(out=phT[:, hi, :], in_=ps)
                        nc.vector.tensor_sub(
                            out=plT[:, hi, :], in0=ps, in1=phT[:, hi, :]
                        )
                    DphT.append(phT)
                    DplT.append(plT)

            # ========== Main loop: per batch, stream E chunks (score + streaming
            # softmax), then finalize softmax and compute context. ==========
            with (
                tc.tile_pool(name="ehp", bufs=1) as ehp,
                tc.tile_pool(name="upool", bufs=4) as upool,
                tc.tile_pool(name="small", bufs=4) as small,
                tc.tile_pool(name="outp", bufs=2) as outp,
            ):
              for b in range(BSH):
                phT, plT = DphT[b], DplT[b]
                ehn = ehp.tile([128, KT, H], f16, tag="ehn", name=f"ehn{b}")
                U = [
                    upool.tile([128, TK], f16, tag="u", name=f"u{b}_{qt}")
                    for qt in range(QT)
                ]
                negm = [
                    small.tile([128, NCH], f32, tag="negm", name=f"negm{b}_{qt}")
                    for qt in range(QT)
                ]
                zz = [
                    small.tile([128, NCH], f32, tag="zz", name=f"zz{b}_{qt}")
                    for qt in range(QT)
                ]
                for c in range(NCH):
                    ehT = ech.tile([128, HT, CK], f16, tag="ehT", name=f"ehT{b}_{c}")
                    elT = ech.tile([128, HT, CK], f16, tag="elT", name=f"elT{b}_{c}")
                    for i in range(CKT):
                        t = c * CKT + i
                        ef = estage.tile([128, H], f32, tag="ef32", name="ef")
                        nc.sync.dma_start(
                            out=ef, in_=enc[b, t * 128 : (t + 1) * 128, :]
                        )
                        nc.scalar.copy(out=ehn[:, t, :], in_=ef)
                        el = estage.tile([128, H], f16, tag="efl", name="el")
                        nc.vector.tensor_sub(out=el, in0=ef, in1=ehn[:, t, :])
                        nc.sync.dma_start_transpose(
                            out=ehT[:, :, i * 128 : (i + 1) * 128], in_=ehn[:, t, :]
                        )
                        nc.sync.dma_start_transpose(
                            out=elT[:, :, i * 128 : (i + 1) * 128], in_=el
                        )
                    for qt in range(QT):
                        ps = sps.tile([128, CK], f32, tag="s", name="s_ps")
                        i = 0
                        for (L, R) in ((phT, ehT), (phT, elT), (plT, ehT)):
                            for hi in range(HT):
                                nc.tensor.matmul(
                                    ps,
                                    L[:, hi, qt * 128 : (qt + 1) * 128],
                                    R[:, hi, :],
                                    start=(i == 0),
                                    stop=(i == 3 * HT - 1),
                                )
                                i += 1
                        nc.vector.reduce_max(
                            out=negm[qt][:, c : c + 1], in_=ps, axis=X, negate=True
                        )
                        nc.scalar.activation(
                            out=U[qt][:, c * CK : (c + 1) * CK],
                            in_=ps,
                            func=Exp,
                            bias=negm[qt][:, c : c + 1],
                            accum_out=zz[qt][:, c : c + 1],
                        )

                # ---------- softmax finalization + context ----------
                for qt in range(QT):
                    negmin = small.tile([128, 1], f32, tag="negmin", name="negmin")
                    nc.vector.tensor_reduce(
                        out=negmin,
                        in_=negm[qt],
                        axis=X,
                        op=mybir.AluOpType.min,
                    )
                    scales = small.tile([128, NCH], f32, tag="scales", name="scales")
                    # exp(-negm + negmin) = exp(m_c - m)
                    nc.scalar.activation(
                        out=scales, in_=negm[qt], func=Exp, bias=negmin, scale=-1.0
                    )
                    zs = small.tile([128, NCH], f32, tag="zs", name="zs")
                    nc.vector.tensor_mul(out=zs, in0=zz[qt], in1=scales)
                    Zt = small.tile([128, 1], f32, tag="Zt", name="Zt")
                    nc.vector.reduce_sum(out=Zt, in_=zs, axis=X)
                    rz = small.tile([128, 1], f32, tag="rz", name="rz")
                    nc.vector.reciprocal(out=rz, in_=Zt)
                    sc = small.tile([128, NCH], f32, tag="sc", name="sc")
                    nc.vector.tensor_scalar_mul(out=sc, in0=scales, scalar1=rz)

                    ab = outp.tile([128, TK], f16, tag="ab", bufs=1, name="ab")
                    for c in range(NCH):
                        ast = outp.tile([128, CK], f32, tag="ast", name="ast")
                        nc.vector.tensor_scalar_mul(
                            out=ast,
                            in0=U[qt][:, c * CK : (c + 1) * CK],
                            scalar1=sc[:, c : c + 1],
                        )
                        nc.sync.dma_start(
                            out=align_o[
                                b, qt * 128 : (qt + 1) * 128, c * CK : (c + 1) * CK
                            ],
                            in_=ast,
                        )
                        nc.scalar.activation(
                            out=ab[:, c * CK : (c + 1) * CK],
                            in_=U[qt][:, c * CK : (c + 1) * CK],
                            func=Copy,
                            scale=sc[:, c : c + 1],
                        )
                    abT = outp.tile([128, KT, 128], f16, tag="abT", bufs=2, name="abT")
                    nc.sync.dma_start_transpose(out=abT, in_=ab)
                    cst = outp.tile([128, H], f32, tag="cst", bufs=2, name="cst")
                    for hc in range(2):
                        cp = cps.tile([128, 512], f32, tag="c", name="c_ps")
                        for t in range(KT):
                            nc.tensor.matmul(
                                cp,
                                abT[:, t, :],
                                ehn[:, t, hc * 512 : (hc + 1) * 512],
                                start=(t == 0),
                                stop=(t == KT - 1),
                            )
                        nc.vector.tensor_copy(
                            out=cst[:, hc * 512 : (hc + 1) * 512], in_=cp
                        )
                    nc.sync.dma_start(
                        out=ctx_o[b, qt * 128 : (qt + 1) * 128, :], in_=cst
                    )

    nc.compile()  # bacc register allocation / DCE; required before walrus codegen
    return nc


def get_program(reps=1):
    key = f"nc{reps}"
    if key not in _CACHE:
        _CACHE[key] = _build_program(reps)
    return _CACHE[key]


def kernel(decoder_output, encoder_output, wa_kernel, wa_bias):
    """Full-input entry point: shards over batch across 8 cores, returns
    (context [B,TQ,H] f32, alignment [B,TQ,TK] f32) like the reference.

    wa_bias only shifts each score row by a per-query constant, which softmax
    is invariant to, so it does not enter the device program.
    """
    from concourse.bass_utils import run_bass_kernel_spmd

    nc = get_program()

    decoder_output = np.ascontiguousarray(decoder_output, dtype=np.float32)
    encoder_output = np.ascontiguousarray(encoder_output, dtype=np.float32)
    wa_kernel = np.ascontiguousarray(wa_kernel, dtype=np.float32)

    in_maps = []
    for core in range(NCORES):
        lo, hi = core * BSH, (core + 1) * BSH
        in_maps.append(
            {
                "dec": decoder_output[lo:hi],
                "enc": encoder_output[lo:hi],
                "wa": wa_kernel,
            }
        )

    res = run_bass_kernel_spmd(nc, in_maps, core_ids=list(range(NCORES)))
    _CACHE["last_results"] = res

    ctx = np.concatenate([res.results[i]["ctx"] for i in range(NCORES)], axis=0)
    align = np.concatenate([res.results[i]["align"] for i in range(NCORES)], axis=0)
    return ctx, align


# revision 39
# speedup vs baseline: 1.4486x; 1.0742x over previous
"""Luong 'general' attention kernel for Trainium2, data-parallel over batch on 8 NeuronCores.

Math (per batch b):
    proj  = enc @ Wa + bias                    [TK, H]
    score = dec @ proj^T                       [TQ, TK]
    align = softmax(score, axis=-1)            [TQ, TK]
    ctx   = align @ enc                        [TQ, H]

Key transformations used here:
  1. The bias term adds a per-query constant to every score row; softmax is
     invariant to it, so it is dropped entirely (it is zeros in this problem
     anyway, and alignment/context are mathematically unchanged for any bias).
  2. score = dec @ (enc @ Wa)^T = (dec @ Wa^T) @ enc^T.  Folding Wa into the
     decoder side costs 2*TQ*H*H flops instead of 2*TK*H*H -- 4x cheaper.
  3. All matmuls run on the PE array in fp16 using 3-pass split precision
     (x = hi + lo, keeping hi@hi + hi@lo + lo@hi) which is ~fp32 accurate at
     3 cycles/row instead of fp32's 4 cycles/row, and enables 16-bit DMA
     transposes (fp32 has no DMA-transpose path).
  4. Softmax over the key dim is chunked (flash-style with chunk-local maxima,
     recombined exactly at the end) so E^T tiles stream through SBUF.
"""

import numpy as np

B, TQ, TK, H = 16, 512, 2048, 1024
NCORES = 8
BSH = B // NCORES  # batches per core

HT = H // 128   # 8  h tiles
QT = TQ // 128  # 4  query tiles
KT = TK // 128  # 16 key tiles
NCH = 4         # key chunks for the streaming softmax
CK = TK // NCH  # 512 keys per chunk
CKT = CK // 128

_CACHE = {}


def _build_program(reps=1):
    import concourse.bass as bass
    import concourse.mybir as mybir
    import concourse.tile as tile
    from concourse import bacc

    f32 = mybir.dt.float32
    f16 = mybir.dt.float16
    Exp = mybir.ActivationFunctionType.Exp
    Copy = mybir.ActivationFunctionType.Copy
    X = mybir.AxisListType.X
    PSUM = bass.MemorySpace.PSUM

    nc = bacc.Bacc(
        "TRN2", target_bir_lowering=False, debug=False, num_devices=NCORES
    )

    dec = nc.dram_tensor("dec", [BSH, TQ, H], f32, kind="ExternalInput")
    enc = nc.dram_tensor("enc", [BSH, TK, H], f32, kind="ExternalInput")
    wa = nc.dram_tensor("wa", [H, H], f32, kind="ExternalInput")
    ctx_o = nc.dram_tensor("ctx", [BSH, TQ, H], f32, kind="ExternalOutput")
    align_o = nc.dram_tensor("align", [BSH, TQ, TK], f32, kind="ExternalOutput")

    with tile.TileContext(nc) as tc:
      for _rep in range(reps):
        with (
            tc.tile_pool(name="dpp", bufs=2) as dpp,
            tc.tile_pool(name="estage", bufs=5) as estage,
            tc.tile_pool(name="ech", bufs=2) as ech,
            tc.tile_pool(name="mmps", bufs=3, space=PSUM) as mmps,
            tc.tile_pool(name="sps", bufs=3, space=PSUM) as sps,
            tc.tile_pool(name="cps", bufs=2, space=PSUM) as cps,
        ):
            # ========== Prologue: D (both batches) + W -> fp16 hi/lo transposed;
            # mm1 (both batches): D'^T = (D @ Wa^T)^T.  Scoped pools so the W/D
            # staging SBUF is reclaimed for the late-phase pools; estage/ech are
            # declared first (low addresses) so batch-0 E streaming overlaps the
            # prologue instead of waiting on its address range. ==========
            DphT, DplT = [], []
            with (
                tc.tile_pool(name="wp", bufs=1) as wp,
                tc.tile_pool(name="dp", bufs=1) as dp,
                tc.tile_pool(name="wds", bufs=4) as wds,
            ):
                # Batch-0 D first: the first mm1 matmul needs all of batch-0's
                # D^T but only W tile 0, so those loads lead the DMA queue; W
                # streams next (mm1 consumes W tile hi at step hi); batch-1 D
                # last.
                dT = {}

                def emit_d(b):
                    dhT = dp.tile([128, HT, TQ], f16, tag="dht", name=f"dhT{b}")
                    dlT = dp.tile([128, HT, TQ], f16, tag="dlt", name=f"dlT{b}")
                    for qt in range(QT):
                        df = wds.tile([128, H], f32, tag="sf32", name="df")
                        nc.sync.dma_start(
                            out=df, in_=dec[b, qt * 128 : (qt + 1) * 128, :]
                        )
                        dh = wds.tile([128, H], f16, tag="sfh", name="dh")
                        nc.scalar.copy(out=dh, in_=df)
                        dl = wds.tile([128, H], f16, tag="sfl", name="dl")
                        nc.vector.tensor_sub(out=dl, in0=df, in1=dh)
                        nc.sync.dma_start_transpose(
                            out=dhT[:, :, qt * 128 : (qt + 1) * 128], in_=dh
                        )
                        nc.sync.dma_start_transpose(
                            out=dlT[:, :, qt * 128 : (qt + 1) * 128], in_=dl
                        )
                    dT[b] = (dhT, dlT)

                emit_d(0)

                WhT = wp.tile([128, HT, H], f16, tag="wht", name="WhT")
                WlT = wp.tile([128, HT, H], f16, tag="wlt", name="WlT")
                for t in range(HT):
                    wf = wds.tile([128, H], f32, tag="sf32", name="wf")
                    nc.sync.dma_start(out=wf, in_=wa[t * 128 : (t + 1) * 128, :])
                    wh = wds.tile([128, H], f16, tag="sfh", name="wh")
                    nc.scalar.copy(out=wh, in_=wf)
                    wl = wds.tile([128, H], f16, tag="sfl", name="wl")
                    nc.vector.tensor_sub(out=wl, in0=wf, in1=wh)
                    nc.sync.dma_start_transpose(
                        out=WhT[:, :, t * 128 : (t + 1) * 128], in_=wh
                    )
                    nc.sync.dma_start_transpose(
                        out=WlT[:, :, t * 128 : (t + 1) * 128], in_=wl
                    )

                n_mm = 3 * HT
                for b in range(BSH):
                    if b > 0:
                        emit_d(b)
                    dhT, dlT = dT[b]
                    phT = dpp.tile([128, HT, TQ], f16, tag="dph", name=f"DphT{b}")
                    plT = dpp.tile([128, HT, TQ], f16, tag="dpl", name=f"DplT{b}")
                    for hi in range(HT):
                        ps = mmps.tile([128, TQ], f32, tag="mm1", name="mm1ps")
                        i = 0
                        for (L, R) in ((WhT, dhT), (WhT, dlT), (WlT, dhT)):
                            for ho in range(HT):
                                nc.tensor.matmul(
                                    ps,
                                    L[:, ho, hi * 128 : (hi + 1) * 128],
                                    R[:, ho, :],
                                    start=(i == 0),
                                    stop=(i == n_mm - 1),
                                )
                                i += 1
                        nc.scalar.copy(out=phT[:, hi, :], in_=ps)
                        nc.vector.tensor_sub(
                            out=plT[:, hi, :], in0=ps, in1=phT[:, hi, :]
                        )
                    DphT.append(phT)
                    DplT.append(plT)

            # ========== Main loop: per batch, stream E chunks (score + streaming
            # softmax), then finalize softmax and compute context. ==========
            with (
                tc.tile_pool(name="ehp", bufs=1) as ehp,
                tc.tile_pool(name="upool", bufs=4) as upool,
                tc.tile_pool(name="small", bufs=4) as small,
                tc.tile_pool(name="outp", bufs=2) as outp,
            ):
              for b in range(BSH):
                phT, plT = DphT[b], DplT[b]
                ehn = ehp.tile([128, KT, H], f16, tag="ehn", name=f"ehn{b}")
                U = [
                    upool.tile([128, TK], f16, tag="u", name=f"u{b}_{qt}")
                    for qt in range(QT)
                ]
                negm = [
                    small.tile([128, NCH], f32, tag="negm", name=f"negm{b}_{qt}")
                    for qt in range(QT)
                ]
                zz = [
                    small.tile([128, NCH], f32, tag="zz", name=f"zz{b}_{qt}")
                    for qt in range(QT)
                ]
                for c in range(NCH):
                    ehT = ech.tile([128, HT, CK], f16, tag="ehT", name=f"ehT{b}_{c}")
                    elT = ech.tile([128, HT, CK], f16, tag="elT", name=f"elT{b}_{c}")
                    for i in range(CKT):
                        t = c * CKT + i
                        ef = estage.tile([128, H], f32, tag="ef32", name="ef")
                        nc.sync.dma_start(
                            out=ef, in_=enc[b, t * 128 : (t + 1) * 128, :]
                        )
                        nc.scalar.copy(out=ehn[:, t, :], in_=ef)
                        el = estage.tile([128, H], f16, tag="efl", name="el")
                        nc.vector.tensor_sub(out=el, in0=ef, in1=ehn[:, t, :])
                        nc.sync.dma_start_transpose(
                            out=ehT[:, :, i * 128 : (i + 1) * 128], in_=ehn[:, t, :]
                        )
                        nc.sync.dma_start_transpose(
                            out=elT[:, :, i * 128 : (i + 1) * 128], in_=el
                        )
                    for qt in range(QT):
                        ps = sps.tile([128, CK], f32, tag="s", name="s_ps")
                        i = 0
                        for (L, R) in ((phT, ehT), (phT, elT), (plT, ehT)):
                            for hi in range(HT):
                                nc.tensor.matmul(
                                    ps,
                                    L[:, hi, qt * 128 : (qt + 1) * 128],
                                    R[:, hi, :],
                                    start=(i == 0),
                                    stop=(i == 3 * HT - 1),
                                )
                                i += 1
                        nc.vector.reduce_max(
                            out=negm[qt][:, c : c + 1], in_=ps, axis=X, negate=True
                        )
                        nc.scalar.activation(
                            out=U[qt][:, c * CK : (c + 1) * CK],
                            in_=ps,
                            func=Exp,
                            bias=negm[qt][:, c : c + 1],
                            accum_out=zz[qt][:, c : c + 1],
                        )

                # ---------- softmax finalization + context ----------
                for qt in range(QT):
                    negmin = small.tile([128, 1], f32, tag="negmin", name="negmin")
                    nc.vector.tensor_reduce(
                        out=negmin,
                        in_=negm[qt],
                        axis=X,
                        op=mybir.AluOpType.min,
                    )
                    scales = small.tile([128, NCH], f32, tag="scales", name="scales")
                    # exp(-negm + negmin) = exp(m_c - m)
                    nc.scalar.activation(
                        out=scales, in_=negm[qt], func=Exp, bias=negmin, scale=-1.0
                    )
                    zs = small.tile([128, NCH], f32, tag="zs", name="zs")
                    nc.vector.tensor_mul(out=zs, in0=zz[qt], in1=scales)
                    Zt = small.tile([128, 1], f32, tag="Zt", name="Zt")
                    nc.vector.reduce_sum(out=Zt, in_=zs, axis=X)
                    rz = small.tile([128, 1], f32, tag="rz", name="rz")
                    nc.vector.reciprocal(out=rz, in_=Zt)
                    sc = small.tile([128, NCH], f32, tag="sc", name="sc")
                    nc.vector.tensor_scalar_mul(out=sc, in0=scales, scalar1=rz)

                    ab = outp.tile([128, TK], f16, tag="ab", bufs=1, name="ab")
                    for c in range(NCH):
                        ast = outp.tile([128, CK], f32, tag="ast", name="ast")
                        nc.vector.tensor_scalar_mul(
                            out=ast,
                            in0=U[qt][:, c * CK : (c + 1) * CK],
                            scalar1=sc[:, c : c + 1],
                        )
                        nc.sync.dma_start(
                            out=align_o[
                                b, qt * 128 : (qt + 1) * 128, c * CK : (c + 1) * CK
                            ],
                            in_=ast,
                        )
                        nc.scalar.activation(
                            out=ab[:, c * CK : (c + 1) * CK],
                            in_=U[qt][:, c * CK : (c + 1) * CK],
                            func=Copy,
                            scale=sc[:, c : c + 1],
                        )
                    abT = outp.tile([128, KT, 128], f16, tag="abT", bufs=2, name="abT")
                    nc.sync.dma_start_transpose(out=abT, in_=ab)
                    cst = outp.tile([128, H], f32, tag="cst", bufs=2, name="cst")
                    for hc in range(2):
                        cp = cps.tile([128, 512], f32, tag="c", name="c_ps")
                        for t in range(KT):
                            nc.tensor.matmul(
                                cp,
                                abT[:, t, :],
                                ehn[:, t, hc * 512 : (hc + 1) * 512],
                                start=(t == 0),
                                stop=(t == KT - 1),
                            )
                        nc.vector.tensor_copy(
                            out=cst[:, hc * 512 : (hc + 1) * 512], in_=cp
                        )
                    nc.sync.dma_start(
                        out=ctx_o[b, qt * 128 : (qt + 1) * 128, :], in_=cst
                    )

    nc.compile()  # bacc register allocation / DCE; required before walrus codegen
    return nc


def get_program(reps=1):
    key = f"nc{reps}"
    if key not in _CACHE:
        _CACHE[key] = _build_program(reps)
    return _CACHE[key]


def kernel(decoder_output, encoder_output, wa_kernel, wa_bias):
    """Full-input entry point: shards over batch across 8 cores, returns
    (context [B,TQ,H] f32, alignment [B,TQ,TK] f32) like the reference.

    wa_bias only shifts each score row by a per-query constant, which softmax
    is invariant to, so it does not enter the device program.
    """
    from concourse.bass_utils import run_bass_kernel_spmd

    nc = get_program()

    decoder_output = np.ascontiguousarray(decoder_output, dtype=np.float32)
    encoder_output = np.ascontiguousarray(encoder_output, dtype=np.float32)
    wa_kernel = np.ascontiguousarray(wa_kernel, dtype=np.float32)

    in_maps = []
    for core in range(NCORES):
        lo, hi = core * BSH, (core + 1) * BSH
        in_maps.append(
            {
                "dec": decoder_output[lo:hi],
                "enc": encoder_output[lo:hi],
                "wa": wa_kernel,
            }
        )

    res = run_bass_kernel_spmd(nc, in_maps, core_ids=list(range(NCORES)))
    _CACHE["last_results"] = res

    ctx = np.concatenate([res.results[i]["ctx"] for i in range(NCORES)], axis=0)
    align = np.concatenate([res.results[i]["align"] for i in range(NCORES)], axis=0)
    return ctx, align
